# revision 1
# baseline (speedup 1.0000x reference)
"""GCN 2-layer kernel for TRN2 x8 cores — host prep + Bass/Tile builder.

Math: out1 = relu(dinv ⊙ (Aᵀ (dinv ⊙ (x@W1))) + b1)
      out2 = relu(dinv ⊙ (Aᵀ (dinv ⊙ out1)) @ W2 + b2)
with A = adjacency incl. self-loops, dinv = rsqrt(in-degree incl self).

Device plan (SPMD, 8 cores, one program):
- host prescales x by dinv, transposes, pads into "table layout"
  (8 blocks of BLK rows; block c = core c's NPC nodes + zero pad rows).
- every core computes full table1 = x~@W1 (node-major bf16, DRAM).
- edge messages fetched with dma_gather (bf16 rows, 256B); segment-sum via
  PE matmuls: lhsT = gathered msgs [128 slots,128 feats] (stationary),
  rhs = fixed block-ones S [128, M] -> accumulate z^T [128 feat, node cols]
  in PSUM banks of 512 node-columns ("windows").
- fixed bucket grid: every node owns B slots per range-stream
  (range A = table rows of cores 0..3, range B = cores 4..7; keeps
  dma_gather's int16 indices in bounds). Overflow edges go to per-window
  "fixup" chunks whose one-hot S is built on device via is_equal(iota, col).
- layer1 tail: h2~ = dinv*relu(dinv*z1+b1) -> transpose -> local block ->
  AllGather -> table2; layer2 same gather/segsum; out2^T = relu(W2^T q^T + b2).
- host assembles final [N, DOUT] from per-core [DOUT, BLK] outputs.
"""
import numpy as np
import ml_dtypes

BF16 = ml_dtypes.bfloat16


# ---------------------------------------------------------------- structure
class Struct:
    pass


def make_structure(N, NC, B, WIN=512):
    P = Struct()
    P.N, P.NC, P.B, P.WIN = N, NC, B, WIN
    assert N % NC == 0
    P.NPC = N // NC
    # block rows: pad NPC up so BLK is a multiple of 32 (window math) and
    # leaves at least one zero pad row (zero-row index for padding slots)
    P.BLK = ((P.NPC + 1 + 31) // 32) * 32
    P.RSPLIT_CORE = NC // 2
    P.RSPLIT = P.RSPLIT_CORE * P.BLK
    P.TROWS = NC * P.BLK
    assert P.RSPLIT <= 32768 and P.TROWS - P.RSPLIT <= 32768
    P.ZLOC = P.NPC  # zero row, block-local (same local idx in both ranges)
    # windows over the BLK node columns
    P.windows = []
    col0 = 0
    while col0 < P.BLK:
        ncols = min(WIN, P.BLK - col0)
        assert (ncols * B) % 128 == 0, (ncols, B)
        P.windows.append(Struct())
        w = P.windows[-1]
        w.col0, w.ncols = col0, ncols
        w.nchunks = ncols * B // 128
        # per-chunk: phase id, node col offset (window-rel), M
        w.chunks = []
        for k in range(w.nchunks):
            s0 = k * 128
            n0 = s0 // B
            M = (s0 + 127) // B - n0 + 1
            lcm = B * 128 // np.gcd(B, 128)
            ph = (s0 % lcm) // 128
            w.chunks.append((ph, n0, M))
        col0 += ncols
    P.NPH = B * 128 // np.gcd(B, 128) // 128  # number of S phase matrices
    # S phase matrices [128, M_p]
    P.S = []
    for p in range(P.NPH):
        n0 = (p * 128) // B
        M = (p * 128 + 127) // B - n0 + 1
        S = np.zeros((128, M), np.float32)
        for e in range(128):
            S[e, (p * 128 + e) // B - n0] = 1.0
        P.S.append(S.astype(BF16))
    return P


# ---------------------------------------------------------------- host prep
def prep(P, x, edge_index, W1, b1, W2, b2):
    """Returns (in_maps, aux) — in_maps is the per-core input dict list."""
    N, NC, B, NPC, BLK = P.N, P.NC, P.B, P.NPC, P.BLK
    F = x.shape[1]
    HID = W1.shape[1]
    DOUT = W2.shape[1]
    P.F, P.HID, P.DOUT = F, HID, DOUT

    src = np.asarray(edge_index[0], np.int64)
    dst = np.asarray(edge_index[1], np.int64)
    deg = np.bincount(dst, minlength=N).astype(np.float64) + 1.0
    dinv = (1.0 / np.sqrt(deg)).astype(np.float32)

    # add self loops
    loops = np.arange(N, dtype=np.int64)
    src = np.concatenate([src, loops])
    dst = np.concatenate([dst, loops])

    # table layout: global node g -> row (g//NPC)*BLK + g%NPC
    row_of = (src // NPC) * BLK + (src % NPC)

    # x~^T in table layout, bf16, [F, TROWS]
    xT = np.zeros((F, P.TROWS), np.float32)
    xs = (x.astype(np.float32) * dinv[:, None]).T  # [F, N]
    for c in range(NC):
        xT[:, c * BLK: c * BLK + NPC] = xs[:, c * NPC:(c + 1) * NPC]
    xT = xT.astype(BF16)

    dst_core = dst // NPC
    dst_local = (dst % NPC).astype(np.int64)
    in_range_b = row_of >= P.RSPLIT
    src_local = np.where(in_range_b, row_of - P.RSPLIT, row_of).astype(np.int64)

    NW = len(P.windows)
    # per (core, stream): bucket array + fixup lists per window
    bucket = np.full((NC, 2, BLK * B), P.ZLOC, np.int64)
    fix = [[[[] for _ in range(NW)] for _ in range(2)] for _ in range(NC)]

    for c in range(NC):
        m = dst_core == c
        dl_c, sl_c, rb_c = dst_local[m], src_local[m], in_range_b[m]
        for s in range(2):
            ms = rb_c == bool(s)
            dl, sl = dl_c[ms], sl_c[ms]
            order = np.argsort(dl, kind="stable")
            dl, sl = dl[order], sl[order]
            # rank within node
            if len(dl):
                starts = np.r_[0, np.flatnonzero(np.diff(dl)) + 1]
                rank = np.arange(len(dl)) - np.repeat(starts, np.diff(np.r_[starts, len(dl)]))
            else:
                rank = np.zeros(0, np.int64)
            inb = rank < B
            bucket[c, s, dl[inb] * B + rank[inb]] = sl[inb]
            ovi = np.flatnonzero(~inb)
            for j in ovi:
                w = int(dl[j]) // P.WIN
                fix[c][s][w].append((int(sl[j]), int(dl[j]) - P.windows[w].col0))

    # fixup capacities (chunks of 128), shared across cores (SPMD)
    P.fixcap = np.zeros((2, NW), np.int64)
    for s in range(2):
        for w in range(NW):
            mx = max(len(fix[c][s][w]) for c in range(NC))
            P.fixcap[s, w] = (mx + 127) // 128
    P.NFIXCH = int(P.fixcap.sum())

    # build per-core slot streams + dstcol array
    # stream s layout: for w: [bucket slots of w] + [fixup chunks of w]
    P.stream_len = [0, 0]
    for s in range(2):
        P.stream_len[s] = sum(
            w.nchunks * 128 + int(P.fixcap[s, wi]) * 128
            for wi, w in enumerate(P.windows)
        )

    def wrap_idxs(flat):
        # [S] -> [128, S/16] int16 (wrapped in 16 partitions, replicated x8)
        Sn = len(flat)
        assert Sn % 16 == 0
        w16 = flat.reshape(Sn // 16, 16).T  # [16, S/16]
        return np.tile(w16, (8, 1)).astype(np.int16)

    in_maps = []
    for c in range(NC):
        streams = []
        dstcols = []
        for s in range(2):
            parts = []
            for wi, w in enumerate(P.windows):
                parts.append(bucket[c, s, w.col0 * B:(w.col0 + w.ncols) * B])
                cap = int(P.fixcap[s, wi]) * 128
                fl = fix[c][s][wi]
                fi = np.full(cap, P.ZLOC, np.int64)
                fcol = np.zeros(cap, np.float32)
                for j, (srow, col) in enumerate(fl):
                    fi[j] = srow
                    fcol[j] = col
                parts.append(fi)
                for ch in range(cap // 128):
                    dstcols.append((s, wi, fcol[ch * 128:(ch + 1) * 128]))
            streams.append(np.concatenate(parts) if parts else np.zeros(0, np.int64))
        # dstcol tensor [128, NFIXCH] fp32 in (stream-major? keep (s,w) order)
        nfix = max(1, P.NFIXCH)
        dc = np.zeros((128, nfix), np.float32)
        # order: same order the builder walks: for w: for s: chunks
        # we stored (s, wi, col); rebuild in builder order:
        bi = 0
        dc_map = {}
        for wi in range(NW):
            for s in range(2):
                for ch in range(int(P.fixcap[s, wi])):
                    dc_map[(s, wi, ch)] = bi
                    bi += 1
        cnt = {}
        for (s, wi, cols) in dstcols:
            ch = cnt.get((s, wi), 0)
            cnt[(s, wi)] = ch + 1
            dc[:, dc_map[(s, wi, ch)]] = cols
        assert bi == P.NFIXCH

        dinvb = np.zeros((128, BLK), np.float32)
        dinvb[:, :NPC] = dinv[c * NPC:(c + 1) * NPC][None, :]

        sall = np.concatenate([np.asarray(S, np.float32) for S in P.S], axis=1)
        iota = np.tile(np.arange(P.WIN, dtype=np.float32), (128, 1))
        ident = np.eye(128, dtype=np.float32)

        in_maps.append({
            "xT": xT,
            "w1": W1.astype(np.float32).astype(BF16),
            "w2": W2.astype(np.float32).astype(BF16),
            "b1": b1.astype(np.float32).reshape(HID, 1),
            "b2": b2.astype(np.float32).reshape(DOUT, 1),
            "dinvb": dinvb,
            "iota": iota,
            "ident": ident.astype(BF16),
            "sall": sall.astype(BF16),
            "idxA": wrap_idxs(streams[0]),
            "idxB": wrap_idxs(streams[1]),
            "dstcol": dc,
        })
    return in_maps


def postprocess(P, results):
    out = np.zeros((P.N, P.DOUT), np.float32)
    for c in range(P.NC):
        blk = results[c]["out"]  # [DOUT, BLK]
        out[c * P.NPC:(c + 1) * P.NPC] = blk[:, :P.NPC].T
    return out


# ---------------------------------------------------------------- builder
def build(P, until='full', parts=('mm','fix','drain','tp'), repeat=1):
    import concourse.bacc as bacc
    import concourse.tile as tile
    import concourse.mybir as mybir
    from concourse import bass

    dt = mybir.dt
    NC, B, BLK, NW = P.NC, P.B, P.BLK, len(P.windows)
    F, HID, DOUT = P.F, P.HID, P.DOUT
    SA16 = P.stream_len[0] // 16
    SB16 = P.stream_len[1] // 16
    NFIX = max(1, P.NFIXCH)
    NT = P.TROWS // 128  # dense tiles

    nc = bacc.Bacc("TRN2", target_bir_lowering=False, debug=False, num_devices=NC)
    xT_d = nc.dram_tensor("xT", [F, P.TROWS], dt.bfloat16, kind="ExternalInput")
    w1_d = nc.dram_tensor("w1", [F, HID], dt.bfloat16, kind="ExternalInput")
    w2_d = nc.dram_tensor("w2", [HID, DOUT], dt.bfloat16, kind="ExternalInput")
    b1_d = nc.dram_tensor("b1", [HID, 1], dt.float32, kind="ExternalInput")
    b2_d = nc.dram_tensor("b2", [DOUT, 1], dt.float32, kind="ExternalInput")
    dinvb_d = nc.dram_tensor("dinvb", [128, BLK], dt.float32, kind="ExternalInput")
    iota_d = nc.dram_tensor("iota", [128, P.WIN], dt.float32, kind="ExternalInput")
    ident_d = nc.dram_tensor("ident", [128, 128], dt.bfloat16, kind="ExternalInput")
    sumM = sum(S.shape[1] for S in P.S)
    sall_d = nc.dram_tensor("sall", [128, sumM], dt.bfloat16, kind="ExternalInput")
    idxA_d = nc.dram_tensor("idxA", [128, SA16], dt.int16, kind="ExternalInput")
    idxB_d = nc.dram_tensor("idxB", [128, SB16], dt.int16, kind="ExternalInput")
    dstcol_d = nc.dram_tensor("dstcol", [128, NFIX], dt.float32, kind="ExternalInput")
    out_d = nc.dram_tensor("out", [DOUT, BLK], dt.float32, kind="ExternalOutput")

    with tile.TileContext(nc) as tc:
        with (
            tc.tile_pool(name="dram", bufs=1, space="DRAM") as dram,
            tc.tile_pool(name="const", bufs=1) as cpool,
            tc.tile_pool(name="xchunk", bufs=3) as xpool,
            tc.tile_pool(name="dcopy", bufs=4) as dcpool,
            tc.tile_pool(name="msgs", bufs=3) as mpool,
            tc.tile_pool(name="sfix", bufs=2) as sfpool,
            tc.tile_pool(name="drain", bufs=3) as drpool,
            tc.tile_pool(name="rows", bufs=3) as rpool,
            tc.tile_pool(name="psum_dense", bufs=2, space="PSUM") as pdense,
            tc.tile_pool(name="psum_z", bufs=2, space="PSUM") as pz,
            tc.tile_pool(name="psum_t", bufs=2, space="PSUM") as pt,
        ):
            table1 = dram.tile([P.TROWS, F], dt.bfloat16)
            ag_in = dram.tile([BLK, HID], dt.bfloat16)
            ag_out = dram.tile([P.TROWS, HID], dt.bfloat16, addr_space="Shared")

            # ---- constants to SBUF
            w1sb = cpool.tile([F, HID], dt.bfloat16)
            nc.sync.dma_start(w1sb[:], w1_d[:])
            w2sb = cpool.tile([HID, DOUT], dt.bfloat16)
            nc.sync.dma_start(w2sb[:], w2_d[:])
            b1sb = cpool.tile([HID, 1], dt.float32)
            nc.sync.dma_start(b1sb[:], b1_d[:])
            b2sb = cpool.tile([DOUT, 1], dt.float32)
            nc.sync.dma_start(b2sb[:], b2_d[:])
            dinvb = cpool.tile([128, BLK], dt.float32)
            nc.sync.dma_start(dinvb[:], dinvb_d[:])
            iota = cpool.tile([128, P.WIN], dt.float32)
            nc.sync.dma_start(iota[:], iota_d[:])
            ident = cpool.tile([128, 128], dt.bfloat16)
            nc.sync.dma_start(ident[:], ident_d[:])
            sall = cpool.tile([128, sumM], dt.bfloat16)
            nc.sync.dma_start(sall[:], sall_d[:])
            soff = np.cumsum([0] + [S.shape[1] for S in P.S])
            idxA = cpool.tile([128, SA16], dt.int16)
            nc.sync.dma_start(idxA[:], idxA_d[:])
            idxB = cpool.tile([128, SB16], dt.int16)
            nc.sync.dma_start(idxB[:], idxB_d[:])
            dstcol = cpool.tile([128, NFIX], dt.float32)
            nc.sync.dma_start(dstcol[:], dstcol_d[:])
            zero512 = cpool.tile([128, P.WIN], dt.bfloat16)
            nc.vector.memset(zero512[:], 0.0)

            # ---- dense: table1 = x~ @ W1  (node-major tiles)
            XC = 8  # tiles per x chunk
            for t0 in range(0, NT, XC):
                ntile = min(XC, NT - t0)
                xc = xpool.tile([128, XC * 128], dt.bfloat16, tag="xc")
                nc.sync.dma_start(
                    xc[:, : ntile * 128], xT_d[:, t0 * 128:(t0 + ntile) * 128]
                )
                for j in range(ntile):
                    t = t0 + j
                    ps = pdense.tile([128, HID], dt.float32, tag="pd")
                    nc.tensor.matmul(
                        ps[:], xc[:, j * 128:(j + 1) * 128], w1sb[:],
                        start=True, stop=True,
                    )
                    h1 = dcpool.tile([128, HID], dt.bfloat16, tag="h1")
                    if j % 2 == 0:
                        nc.vector.tensor_copy(h1[:], ps[:])
                    else:
                        nc.scalar.copy(h1[:], ps[:])
                    nc.sync.dma_start(table1[t * 128:(t + 1) * 128, :], h1[:])

            # ---- edge phase (shared for both layers)
            def edge_layer(table, layer):
                tabA = table[0:P.RSPLIT, :]
                tabB = table[P.RSPLIT:P.TROWS, :]
                offs = [0, 0]  # slot offsets (in 16-col units) per stream
                fix_i = 0
                for wi, w in enumerate(P.windows):
                    zw = pz.tile([128, P.WIN], dt.float32, tag="z")
                    nc.tensor.matmul(
                        zw[:, 0:P.WIN], zero512[:, :128], zero512[:],
                        start=True, stop=False,
                    )

                    def mm(out_ap, lhsT, rhs, last=False):
                        nc.tensor.matmul(
                            out_ap, lhsT, rhs, start=False, stop=last
                        )

                    nmm_total = (
                        w.nchunks * 2
                        + int(P.fixcap[0, wi]) + int(P.fixcap[1, wi])
                    )
                    nmm = 0
                    for s in range(2):
                        idx = idxA if s == 0 else idxB
                        tab = tabA if s == 0 else tabB
                        ncall = w.nchunks + int(P.fixcap[s, wi])
                        nslots = ncall * 128
                        msgs = mpool.tile([128, ncall, F], dt.bfloat16, tag="m")
                        nc.gpsimd.dma_gather(
                            msgs[:], tab, idx[:, offs[s]: offs[s] + nslots // 16],
                            nslots, nslots, F, single_packet=False,
                        )
                        offs[s] += nslots // 16
                        for k in range(w.nchunks):
                            ph, n0, M = w.chunks[k]
                            nmm += 1
                            if 'mm' not in parts:
                                continue
                            mm(
                                zw[:, n0:n0 + M],
                                msgs[:, k, :],
                                sall[:, int(soff[ph]):int(soff[ph]) + M],
                                last=(nmm == nmm_total),
                            )
                        for ch in range(int(P.fixcap[s, wi])):
                            if 'fix' not in parts:
                                nmm += 1
                                continue
                            sf = sfpool.tile([128, P.WIN], dt.bfloat16, tag="sf")
                            nc.vector.tensor_scalar(
                                sf[:], iota[:], dstcol[:, fix_i + ch: fix_i + ch + 1],
                                None, op0=mybir.AluOpType.is_equal,
                            )
                            nmm += 1
                            mm(
                                zw[:, 0:P.WIN],
                                msgs[:, w.nchunks + ch, :],
                                sf[:],
                                last=(nmm == nmm_total),
                            )
                        fix_i += int(P.fixcap[s, wi])
                    cols = slice(w.col0, w.col0 + w.ncols)
                    ncols = w.ncols
                    if 'drain' not in parts:
                        continue
                    if layer == 1:
                        t1 = drpool.tile([128, P.WIN], dt.float32, tag="t1")
                        nc.vector.tensor_tensor(
                            t1[:, :ncols], zw[:, :ncols], dinvb[:, cols],
                            op=mybir.AluOpType.mult,
                        )
                        t2 = drpool.tile([128, P.WIN], dt.float32, tag="t2")
                        nc.scalar.activation(
                            t2[:, :ncols], t1[:, :ncols],
                            mybir.ActivationFunctionType.Relu, bias=b1sb[:],
                        )
                        h2b = drpool.tile([128, P.WIN], dt.bfloat16, tag="h2b")
                        nc.vector.tensor_tensor(
                            h2b[:, :ncols], t2[:, :ncols], dinvb[:, cols],
                            op=mybir.AluOpType.mult,
                        )
                        for j in range(0, ncols, 128):
                            if 'tp' not in parts:
                                continue
                            nj = min(128, ncols - j)
                            tp = pt.tile([128, 128], dt.bfloat16, tag="tp")
                            nc.tensor.transpose(
                                tp[:nj, :], h2b[:, j:j + nj], ident[:]
                            )
                            hr = rpool.tile([128, 128], dt.bfloat16, tag="hr")
                            nc.vector.tensor_copy(hr[:nj, :], tp[:nj, :])
                            nc.sync.dma_start(
                                ag_in[w.col0 + j: w.col0 + j + nj, :], hr[:nj, :]
                            )
                    else:
                        qT = drpool.tile([128, P.WIN], dt.bfloat16, tag="qT")
                        nc.vector.tensor_tensor(
                            qT[:, :ncols], zw[:, :ncols], dinvb[:, cols],
                            op=mybir.AluOpType.mult,
                        )
                        po = pt.tile([DOUT, P.WIN], dt.float32, tag="po")
                        nc.tensor.matmul(
                            po[:, :ncols], w2sb[:], qT[:, :ncols],
                            start=True, stop=True,
                        )
                        ot = rpool.tile([DOUT, P.WIN], dt.float32, tag="ot")
                        nc.scalar.activation(
                            ot[:, :ncols], po[:, :ncols],
                            mybir.ActivationFunctionType.Relu, bias=b2sb[:],
                        )
                        nc.sync.dma_start(out_d[:, cols], ot[:, :ncols])

            if repeat > 1 and until == 'full':
                with tc.For_i(0, repeat, 1) as _i:
                    edge_layer(table1, 1)
                    edge_layer(table1, 2)
            elif until in ('l1', 'ag', 'full'):
                edge_layer(table1, 1)
            if until == 'agloop':
                for _r in range(repeat):
                    ago_r = dram.tile(
                        [P.TROWS, HID], dt.bfloat16, addr_space="Shared",
                        name=f"ago{_r}",
                    )
                    nc.gpsimd.collective_compute(
                        "AllGather",
                        mybir.AluOpType.bypass,
                        ins=[ag_in.opt()],
                        outs=[ago_r.opt()],
                        replica_groups=[list(range(NC))],
                    )
            if until in ('ag', 'full') and repeat == 1:
                nc.gpsimd.collective_compute(
                    "AllGather",
                    mybir.AluOpType.bypass,
                    ins=[ag_in.opt()],
                    outs=[ag_out.opt()],
                    replica_groups=[list(range(NC))],
                )
            if until == 'full' and repeat == 1:
                edge_layer(ag_out, 2)
            elif until != 'full':
                zo = cpool.tile([DOUT, BLK], dt.float32)
                nc.vector.memset(zo[:], 0.0)
                nc.sync.dma_start(out_d[:], zo[:])

    nc.compile()
    return nc


# ----------------------------------------------------------------- kernel()
# Self-contained entry point: takes FULL inputs, shards across 8 NeuronCores
# internally (SPMD, one program), returns the FULL [N, DOUT] output.
_BUILD_CACHE = {}


def _get_nc(P, key, **bkw):
    ent = _BUILD_CACHE.get(key)
    if ent is None:
        ent = build(P, **bkw)
        _BUILD_CACHE[key] = ent
    return ent


def kernel(x, edge_index, W1, b1, W2, b2):
    import numpy as np
    x = np.asarray(x)
    edge_index = np.asarray(edge_index)
    N = x.shape[0]
    NC = 8
    P = make_structure(N, NC, B=12)
    in_maps = prep(P, x, edge_index, np.asarray(W1), np.asarray(b1),
                   np.asarray(W2), np.asarray(b2))
    key = (N, x.shape[1], np.asarray(W2).shape[1],
           tuple(P.fixcap.flatten().tolist()))
    nc = _get_nc(P, key)
    from concourse.bass_utils import run_bass_kernel_spmd
    res = run_bass_kernel_spmd(nc, in_maps, core_ids=list(range(NC)))
    return postprocess(P, res.results).astype(np.float32)


def measure_hw_time_ns(x, edge_index, W1, b1, W2, b2, reps=(1, 9), agreps=(2, 18), trials=3):
    """Estimate on-device exec time of one full kernel invocation.

    No NTFF profiling is available through this PJRT/axon path, so we
    amplify: run a build with the edge phases iterated K times on-device
    (tc.For_i) and diff wall-clock against K=1; the AllGather (which cannot
    sit inside control flow) is amplified separately with unrolled copies.
    Host->device transfer and dispatch overhead cancel in the diffs.
    Returns (total_ns, details).
    """
    import time
    import numpy as np
    from concourse.bass_utils import run_bass_kernel_spmd
    x = np.asarray(x)
    N = x.shape[0]
    NC = 8
    P = make_structure(N, NC, B=12)
    in_maps = prep(P, x, edge_index, np.asarray(W1), np.asarray(b1),
                   np.asarray(W2), np.asarray(b2))
    base = (N, x.shape[1], np.asarray(W2).shape[1],
            tuple(P.fixcap.flatten().tolist()))

    def wall(**bkw):
        nc = _get_nc(P, base + tuple(sorted(bkw.items())), **bkw)
        ts = []
        for _ in range(trials):
            t0 = time.perf_counter()
            run_bass_kernel_spmd(nc, in_maps, core_ids=list(range(NC)))
            ts.append(time.perf_counter() - t0)
        return min(ts)

    t_lo = wall(repeat=reps[0])
    t_hi = wall(repeat=reps[1])
    edge_ns = max(0.0, (t_hi - t_lo)) / (reps[1] - reps[0]) * 1e9
    a_lo = wall(until='agloop', repeat=agreps[0])
    a_hi = wall(until='agloop', repeat=agreps[1])
    ag_ns = max(0.0, (a_hi - a_lo)) / (agreps[1] - agreps[0]) * 1e9
    # dense phase isn't amplified; estimate from model (small: ~100us incl DMA)
    dense_ns = 100e3
    total = edge_ns + ag_ns + dense_ns
    return total, {"edge_ns": edge_ns, "ag_ns": ag_ns, "dense_ns(model)": dense_ns}

